# revision 5
# baseline (speedup 1.0000x reference)
"""DeltaNet (decaying-state linear attention) Trainium2 Bass kernel.

Problem: B=4, S=4096, H=1024, NH=16, HD=64.
  q,k = phi(x@W^T) (phi = elu+1), v = x@Wv^T
  beta = clip(sigmoid(x@Wb^T + bb + slope*tau), 0.9, 0.9995)  per (b,s,head)
  scan over s: S_t = beta*S_{t-1} + k v^T ; Z_t = beta*Z + k
               y_t = (q.S_t) / (q.Z_t + eps)
  out = RMSNorm(x + y@Wo^T + bo) * ln_w

Strategy (8 cores):
  Launch 1 — head-parallel: core c owns heads (2c, 2c+1) => 8 (b,head) states.
    The scan is chunked (C=128): with L = cumsum(log beta) within a chunk,
    P = exp(L), R = exp(L_C - L):
      A^T = (k/P)(qP)^T  (masked t2<=t1),  numden = (qP)@Sext + tril(A)@[v|1]
      Sext' = P_C * Sext + (kR)^T @ [v|1]
    Projections in bf16 (fp32 accum in PSUM); x transposed on the fly via
    SWDGE cast-DMA (fp32->bf16) + HWDGE dma transpose.  Outputs y^T (bf16).
  Launch 2 — token-parallel: core c owns 2048 tokens: out-projection +
    residual + RMSNorm.
"""

import numpy as np
import ml_dtypes

import concourse.bass as bass
import concourse.tile as tile
from concourse import bacc, mybir
from concourse.bass_utils import run_bass_kernel_spmd

F32 = mybir.dt.float32
BF16 = mybir.dt.bfloat16
AF = mybir.ActivationFunctionType
OP = mybir.AluOpType

B, S, H, NH, HD = 4, 4096, 1024, 16, 64
NCORES = 8
HPC = NH // NCORES          # heads per core = 2
C = 128                     # chunk length
NCH = S // C                # 32 chunks
TOK = B * S                 # 16384
TPC = TOK // NCORES         # tokens per core (launch 2) = 2048
EPS = 1e-6
NPROJ = 3 * HPC * HD + HPC  # 386 projection outputs per core


def _np_bf16(a):
    return np.asarray(a, dtype=ml_dtypes.bfloat16)


# ---------------------------------------------------------------- launch 1

def build_launch1():
    nc = bacc.Bacc("TRN2", target_bir_lowering=False, debug=False,
                   num_devices=NCORES)

    x_d = nc.dram_tensor("x", [B, S, H], F32, kind="ExternalInput")
    wt_d = nc.dram_tensor("wt", [8, 128, NPROJ], BF16, kind="ExternalInput")
    bgate_d = nc.dram_tensor("bgate", [128, NCH, HPC], F32, kind="ExternalInput")
    maskc_d = nc.dram_tensor("maskc", [128, NCH, B], F32, kind="ExternalInput")
    yT_d = nc.dram_tensor("yT", [128, TOK], BF16, kind="ExternalOutput")

    triu_d = nc.inline_tensor(
        np.asarray(np.arange(128)[:, None] <= np.arange(128)[None, :],
                   dtype=np.float32), name="triu")
    identb_d = nc.inline_tensor(np.eye(128, dtype=ml_dtypes.bfloat16),
                                name="identb")
    e127 = np.zeros((128, 128), np.float32)
    e127[127, :] = 1.0
    e127_d = nc.inline_tensor(e127, name="e127")

    with tile.TileContext(nc) as tc, \
            tc.tile_pool(name="const", bufs=1) as const, \
            tc.tile_pool(name="load", bufs=2) as loadp, \
            tc.tile_pool(name="work", bufs=2) as workp, \
            tc.tile_pool(name="ps", bufs=1, space="PSUM") as psp:
        wt_s = const.tile([128, 8, NPROJ], BF16)
        nc.sync.dma_start(wt_s[:], wt_d.ap().rearrange("a p f -> p a f"))
        triu_s = const.tile([128, 128], F32)
        nc.sync.dma_start(triu_s[:], triu_d[:])
        identb_s = const.tile([128, 128], BF16)
        nc.sync.dma_start(identb_s[:], identb_d[:])
        e127_s = const.tile([128, 128], F32)
        nc.sync.dma_start(e127_s[:], e127_d[:])
        bgate_s = const.tile([128, NCH, HPC], F32)
        nc.sync.dma_start(bgate_s[:], bgate_d[:])
        maskc_s = const.tile([128, NCH, B], F32)
        nc.sync.dma_start(maskc_s[:], maskc_d[:])
        # persistent scan state: [64, pair, 65] fp32, pair = b*HPC + h
        state = const.tile([64, B * HPC, HD + 1], F32)
        nc.vector.memset(state[:], 0.0)

        if True:
            for ch in range(NCH):
                for b in range(B):
                    mcol = maskc_s[:, ch, b:b + 1]
                    # ---- load + transpose x chunk
                    xbf = loadp.tile([128, H], BF16, tag="xbf", bufs=2)
                    nc.gpsimd.dma_start(xbf[:], x_d.ap()[b, ch * C:(ch + 1) * C, :])
                    xT = loadp.tile([128, 8, 128], BF16, tag="xT", bufs=2)
                    for kt in range(8):
                        nc.sync.dma_start_transpose(
                            xT[:, kt, :], xbf[:, kt * 128:(kt + 1) * 128])
                    # ---- projections: [128 tok, NPROJ] fp32 in PSUM
                    proj = psp.tile([128, NPROJ], F32, tag="proj", bufs=2)
                    for kt in range(8):
                        nc.tensor.matmul(proj[:], xT[:, kt, :], wt_s[:, kt, :],
                                         start=(kt == 0), stop=(kt == 7))
                    # ---- phi(q|k) = relu(u) + exp(min(u,0)), [128, 256]
                    qk = proj[:, 0:2 * HPC * HD]
                    t_relu = workp.tile([128, 2 * HPC * HD], F32, tag="phr", bufs=2)
                    nc.scalar.activation(t_relu[:], qk, AF.Relu)
                    t_min = workp.tile([128, 2 * HPC * HD], F32, tag="phm", bufs=2)
                    nc.vector.tensor_scalar_min(t_min[:], qk, 0.0)
                    t_exp = workp.tile([128, 2 * HPC * HD], F32, tag="phe", bufs=2)
                    nc.scalar.activation(t_exp[:], t_min[:], AF.Exp)
                    qphi = workp.tile([128, 2 * HPC * HD], F32, tag="qphi", bufs=2)
                    nc.vector.tensor_tensor(qphi[:], t_relu[:], t_exp[:], OP.add)
                    # ---- beta -> log cumsum -> P, Pinv, R
                    blog = workp.tile([128, HPC], F32, tag="blog", bufs=2)
                    nc.vector.tensor_tensor(blog[:], proj[:, 3 * HPC * HD:NPROJ],
                                            bgate_s[:, ch, :], OP.add)
                    bsig = workp.tile([128, HPC], F32, tag="bsig", bufs=2)
                    nc.scalar.activation(bsig[:], blog[:], AF.Sigmoid)
                    bclip = workp.tile([128, HPC], F32, tag="bclip", bufs=2)
                    nc.vector.tensor_scalar(bclip[:], bsig[:], 0.9, 0.9995,
                                            OP.max, OP.min)
                    omm = workp.tile([128, 1], F32, tag="omm", bufs=2)
                    nc.vector.tensor_scalar(omm[:], mcol, -1.0, 1.0,
                                            OP.mult, OP.add)
                    beff = workp.tile([128, HPC], F32, tag="beff", bufs=2)
                    for h in range(HPC):
                        nc.vector.scalar_tensor_tensor(
                            beff[:, h:h + 1], bclip[:, h:h + 1], mcol, omm[:],
                            OP.mult, OP.add)
                    lb = workp.tile([128, HPC], F32, tag="lb", bufs=2)
                    nc.scalar.activation(lb[:], beff[:], AF.Ln)
                    lps = psp.tile([128, 4], F32, tag="lpb", bufs=1)
                    nc.tensor.matmul(lps[:, 0:HPC], triu_s[:], lb[:])
                    p_s = workp.tile([128, HPC], F32, tag="p_s", bufs=2)
                    nc.scalar.activation(p_s[:], lps[:, 0:HPC], AF.Exp)
                    pinv = workp.tile([128, HPC], F32, tag="pinv", bufs=2)
                    nc.scalar.activation(pinv[:], lps[:, 0:HPC], AF.Exp, scale=-1.0)
                    nc.tensor.matmul(lps[:, 2:2 + HPC], e127_s[:], p_s[:])
                    pb = workp.tile([128, HPC], F32, tag="pb", bufs=2)
                    nc.vector.tensor_copy(pb[:], lps[:, 2:2 + HPC])
                    sk1 = workp.tile([128, HPC], F32, tag="sk1", bufs=2)
                    nc.vector.tensor_scalar_mul(sk1[:], pinv[:], mcol)
                    r_s = workp.tile([128, HPC], F32, tag="r_s", bufs=2)
                    nc.vector.tensor_tensor(r_s[:], pinv[:], pb[:], OP.mult)
                    sk2 = workp.tile([128, HPC], F32, tag="sk2", bufs=2)
                    nc.vector.tensor_scalar_mul(sk2[:], r_s[:], mcol)

                    ypack = workp.tile([128, 128], BF16, tag="ypack", bufs=2)
                    for h in range(HPC):
                        pair = b * HPC + h
                        qsl = qphi[:, h * HD:(h + 1) * HD]
                        ksl = qphi[:, HPC * HD + h * HD:HPC * HD + (h + 1) * HD]
                        vsl = proj[:, 2 * HPC * HD + h * HD:2 * HPC * HD + (h + 1) * HD]
                        # scaled q~, k~ (bf16) and their transposes
                        qpk = workp.tile([128, HD], BF16, tag="qpk", bufs=2)
                        nc.vector.tensor_scalar_mul(qpk[:], qsl, p_s[:, h:h + 1])
                        kpk = workp.tile([128, HD], BF16, tag="kpk", bufs=2)
                        nc.vector.tensor_scalar_mul(kpk[:], ksl, sk1[:, h:h + 1])
                        qT_ps = psp.tile([64, 128], BF16, tag="qkT", bufs=2)
                        nc.tensor.transpose(qT_ps[:], qpk[:], identb_s[:])
                        kT_ps = psp.tile([64, 128], BF16, tag="qkT", bufs=2)
                        nc.tensor.transpose(kT_ps[:], kpk[:], identb_s[:])
                        qT = workp.tile([64, 128], BF16, tag="qT", bufs=2)
                        nc.scalar.copy(qT[:], qT_ps[:])
                        kT = workp.tile([64, 128], BF16, tag="kT", bufs=2)
                        nc.vector.tensor_copy(kT[:], kT_ps[:])
                        # A^T = k~ . q~^T   [t2, t1]
                        a_ps = psp.tile([128, 128], F32, tag="a", bufs=1)
                        nc.tensor.matmul(a_ps[:], kT[:], qT[:])
                        mA = workp.tile([128, 128], BF16, tag="mA", bufs=2)
                        nc.vector.tensor_tensor(mA[:], a_ps[:], triu_s[:], OP.mult)
                        # v_ext = [v*m | 1]
                        v_ext = workp.tile([128, HD + 1], BF16, tag="vext", bufs=2)
                        nc.vector.tensor_scalar_mul(v_ext[:, 0:HD], vsl, mcol)
                        nc.gpsimd.memset(v_ext[:, HD:HD + 1], 1.0)
                        k2 = workp.tile([128, HD], BF16, tag="k2", bufs=2)
                        nc.vector.tensor_scalar_mul(k2[:], ksl, sk2[:, h:h + 1])
                        sb = workp.tile([64, HD + 1], BF16, tag="sb", bufs=2)
                        nc.scalar.copy(sb[:], state[:, pair, :])
                        # numden = q~ @ Sext + tril(A) @ v_ext
                        nd = psp.tile([128, HD + 1], F32, tag="nd", bufs=1)
                        nc.tensor.matmul(nd[:], qT[:], sb[:], start=True, stop=False)
                        nc.tensor.matmul(nd[:], mA[:], v_ext[:], start=False, stop=True)
                        # KV update + state
                        kv = psp.tile([64, HD + 1], F32, tag="kv", bufs=1)
                        nc.tensor.matmul(kv[:], k2[:], v_ext[:])
                        nc.vector.scalar_tensor_tensor(
                            state[:, pair, :], state[:, pair, :],
                            pb[0:64, h:h + 1], kv[:], OP.mult, OP.add)
                        # y = num / (den + eps)
                        rec = workp.tile([128, 1], F32, tag="rec", bufs=2)
                        nc.vector.tensor_scalar_add(rec[:], nd[:, HD:HD + 1], EPS)
                        nc.vector.reciprocal(rec[:], rec[:])
                        nc.vector.tensor_scalar_mul(
                            ypack[:, h * HD:(h + 1) * HD], nd[:, 0:HD], rec[:])
                    # ---- y^T out
                    yTb = workp.tile([128, 128], BF16, tag="yTb", bufs=2)
                    nc.sync.dma_start_transpose(yTb[:], ypack[:])
                    nc.sync.dma_start(
                        yT_d.ap()[:, b * S + ch * C: b * S + (ch + 1) * C], yTb[:])
    nc.compile()
    return nc


# ---------------------------------------------------------------- launch 2

def build_launch2():
    nc = bacc.Bacc("TRN2", target_bir_lowering=False, debug=False,
                   num_devices=NCORES)
    yt_d = nc.dram_tensor("yt", [H, TPC], BF16, kind="ExternalInput")
    x2_d = nc.dram_tensor("x2", [TPC, H], F32, kind="ExternalInput")
    wo_d = nc.dram_tensor("wo", [8, 128, H], BF16, kind="ExternalInput")
    bl_d = nc.dram_tensor("bl", [2, H], F32, kind="ExternalInput")
    o_d = nc.dram_tensor("o", [TPC, H], F32, kind="ExternalOutput")

    NT = TPC // 128  # 16 chunks

    with tile.TileContext(nc) as tc:
        with (
            tc.tile_pool(name="const2", bufs=1) as const,
            tc.tile_pool(name="work2", bufs=2) as workp,
            tc.tile_pool(name="ps2", bufs=2, space="PSUM") as psp,
        ):
            yt_s = const.tile([128, 8, TPC], BF16)
            nc.sync.dma_start(yt_s[:], yt_d.ap().rearrange("(a p) t -> p a t", p=128))
            wo_s = const.tile([128, 8, H], BF16)
            nc.sync.dma_start(wo_s[:], wo_d.ap().rearrange("a p f -> p a f"))
            bo_bf = const.tile([1, H], BF16)
            nc.gpsimd.dma_start(bo_bf[:], bl_d.ap()[0:1, :])
            lnw_row = const.tile([1, H], F32)
            nc.sync.dma_start(lnw_row[:], bl_d.ap()[1:2, :])
            lnw_b = const.tile([128, H], F32)
            nc.gpsimd.partition_broadcast(lnw_b[:], lnw_row[:])
            ones_bf = const.tile([1, 128], BF16)
            nc.vector.memset(ones_bf[:], 1.0)

            for t in range(NT):
                xc = workp.tile([128, H], F32, tag="xc", bufs=3)
                nc.sync.dma_start(xc[:], x2_d.ap()[t * 128:(t + 1) * 128, :])
                op_ps = psp.tile([128, H], F32, tag="op", bufs=2)
                for nh in range(2):
                    osl = op_ps[:, nh * 512:(nh + 1) * 512]
                    for kt in range(8):
                        nc.tensor.matmul(
                            osl, yt_s[:, kt, t * 128:(t + 1) * 128],
                            wo_s[:, kt, nh * 512:(nh + 1) * 512],
                            start=(kt == 0), stop=False)
                    nc.tensor.matmul(osl, ones_bf[:],
                                     bo_bf[:, nh * 512:(nh + 1) * 512],
                                     start=False, stop=True)
                res = workp.tile([128, H], F32, tag="res", bufs=2)
                nc.vector.tensor_tensor(res[:], op_ps[:], xc[:], OP.add)
                sq = workp.tile([128, H], F32, tag="sq", bufs=2)
                ss = workp.tile([128, 1], F32, tag="ss", bufs=2)
                nc.scalar.activation(sq[:], res[:], AF.Square, accum_out=ss[:])
                rms = workp.tile([128, 1], F32, tag="rms", bufs=2)
                nc.scalar.activation(rms[:], ss[:], AF.Sqrt, scale=1.0 / H)
                rec = workp.tile([128, 1], F32, tag="rec2", bufs=2)
                nc.vector.tensor_scalar_add(rec[:], rms[:], EPS)
                nc.vector.reciprocal(rec[:], rec[:])
                fin = workp.tile([128, H], F32, tag="fin", bufs=2)
                nc.vector.scalar_tensor_tensor(fin[:], res[:], rec[:], lnw_b[:],
                                               OP.mult, OP.mult)
                nc.sync.dma_start(o_d.ap()[t * 128:(t + 1) * 128, :], fin[:])
    nc.compile()
    return nc


# ---------------------------------------------------------------- host glue

_CACHE = {}


def _host_prep_launch1(inputs):
    x = np.ascontiguousarray(inputs["x"], dtype=np.float32)
    mask = np.asarray(inputs["mask"], dtype=np.float32)
    Wq, Wk, Wv = (np.asarray(inputs[k], np.float32) for k in ("Wq", "Wk", "Wv"))
    Wb = np.asarray(inputs["Wb"], np.float32)
    bb = np.asarray(inputs["bb"], np.float32)
    slope = np.asarray(inputs["slope"], np.float32)

    tau = np.arange(S, dtype=np.float32) / max(S - 1, 1)
    maskc = np.ascontiguousarray(
        mask.reshape(B, NCH, C).transpose(2, 1, 0))            # [128, NCH, B]
    in_maps = []
    for c in range(NCORES):
        rs = slice(c * HPC * HD, (c + 1) * HPC * HD)
        hs = slice(c * HPC, (c + 1) * HPC)
        cat = np.concatenate([Wq[rs], Wk[rs], Wv[rs], Wb[hs]], axis=0)  # [386, H]
        wt = _np_bf16(np.ascontiguousarray(cat.T).reshape(8, 128, NPROJ))
        bg = (bb[hs][None, None, :] +
              slope[hs][None, None, :] * tau.reshape(NCH, C, 1).transpose(1, 0, 2))
        in_maps.append({
            "x": x,
            "wt": wt,
            "bgate": np.ascontiguousarray(bg, np.float32),
            "maskc": maskc,
        })
    return in_maps


def kernel(**inputs):
    if "l1" not in _CACHE:
        _CACHE["l1"] = build_launch1()
        _CACHE["l2"] = build_launch2()
    nc1, nc2 = _CACHE["l1"], _CACHE["l2"]

    in_maps1 = _host_prep_launch1(inputs)
    r1 = run_bass_kernel_spmd(nc1, in_maps1, core_ids=list(range(NCORES)))
    yT = np.concatenate([r1.results[c]["yT"] for c in range(NCORES)], axis=0)

    x = np.asarray(inputs["x"], np.float32).reshape(TOK, H)
    wo = _np_bf16(np.ascontiguousarray(np.asarray(inputs["Wo"], np.float32).T
                                       ).reshape(8, 128, H))
    bl = np.ascontiguousarray(
        np.stack([np.asarray(inputs["bo"], np.float32),
                  np.asarray(inputs["ln_w"], np.float32)]))
    in_maps2 = []
    for c in range(NCORES):
        ts = slice(c * TPC, (c + 1) * TPC)
        in_maps2.append({
            "yt": np.ascontiguousarray(yT[:, ts]),
            "x2": np.ascontiguousarray(x[ts]),
            "wo": wo,
            "bl": bl,
        })
    r2 = run_bass_kernel_spmd(nc2, in_maps2, core_ids=list(range(NCORES)))
    out = np.concatenate([r2.results[c]["o"] for c in range(NCORES)], axis=0)
    return np.ascontiguousarray(out.reshape(B, S, H))
